# revision 48
# baseline (speedup 1.0000x reference)
"""Trainium2 Bass kernel for the CRF + cross-entropy loss bundle (v3).

loss1 = CRF NLL over emissions [B,S,T=3]; loss2 = entity CE ([B*32,4],
ignore_index=0); loss3 = intent CE [B,10]; out = [mean, l1, l2, l3].
Data-parallel over B=4096 -> 512 samples/core on 8 cores; per-core
partial sums return in 16 f32 accumulator columns, combined on host.

Denominator: absorbing-state chunked linear-space scan (64 chunks of
L=8 transitions, W=1 warm step; validated <1e-4). Per step and chain:
one PE matmul with the block-diagonal absorbing transfer matrix and one
DVE multiply by exp(x-kappa). The xs plane is stored as two contiguous
blocks (chain A cols [0:264) per q-block, chain B cols [256:520), 4 pad
cols) so chain A starts scanning ~3.5us before chain B's DMA lands; the
two chains then interleave with LAG=2 on a double-buffered PSUM set.
Telescoping readout: DEN = sum ln(S_end) - sum_{c>=1} ln(S_warm)
+ kappa*sum(len); warm block-sums go to one [128,1024] PSUM pair whose
deferred Ln runs inside the exp stream; chain-A's end readout overlaps
chain-B's last step.

Numerator: ALL label/mask-only terms (transition score, start/end,
mask sums, CE valid counts) are computed on host in float64. Device
work is only the emission gather q = sum_t em[lbl_t, t] =
sum(mask*em0) + sum(oh1*d1) + sum(oh2*d2) with ohk = [lbl>=k]*mask:
the oh*d products run on GPSIMD in 4 chunks pipelined against their
own DMA; the plane sums run on DVE as 4x-mode tensor_scalar
accumulates: the xs-ch0 sums fill pre-scan/early-scan DVE idle, while
the oh*d plane sums write their (unused) outputs onto EHF[8] so the
WAR against the last scan multiply keeps them behind it -- the final
Ln's chain shortens without growing the DVE total (APs exclude the
pad/overlap cols so the host -40 correction stays exact).

CE: exps on ScalarE; the entity class-quad reduce runs on GPSIMD as 3
strided adds; invalid entity rows have logits zeroed on host so their
lse is exactly ln(4), subtracted on host (no valid-mask multiply on
device). Gathers are fused product+accumulates on DVE.

Scheduling notes (tile-framework semantics): cross-engine dependencies
resolve to per-engine counting-semaphore positions, and any PSUM write
emitted between a matmul and its consumer lands in the consumer's wait
chain -- so emission order tracks intended execution time everywhere,
readout/CE ops are emitted post-loop and rely on engine run-ahead, and
nothing else shares the PE stream with the scan matmuls (an earlier
PE-based ones-matmul reduction of the emission planes saturated PE at
the cold p-state and paced the whole scan).
"""
import math
import numpy as np
import ml_dtypes

import concourse.bass as bass
import concourse.mybir as mybir
from concourse import tile
from concourse.bass_utils import run_bass_kernel_spmd

F32 = mybir.dt.float32
BF16 = mybir.dt.bfloat16
AL = mybir.AluOpType
AF = mybir.ActivationFunctionType
AX = mybir.AxisListType
BF = ml_dtypes.bfloat16

NCORES = 8
B, S, T = 4096, 512, 3
BS = B // NCORES
G = BS // 128            # natural-layout groups (4)
C, L, WU = 64, 8, 1      # chunks, chunk len, warmup (dual chain)
NSTEP = L + WU           # 9
W = 264                  # xs block width per q (260 real + 4 pad)
XA = 16 * W              # chain block size (4224)
U = 512                  # scan free size per chain (16 q x 32 c)
SP = 520                 # padded time width for scan planes
KAPPA = math.log(3.0) + 0.5
NACC = 16
NPW = 4 * G * S          # d1 | oh1 | d2 | oh2
CEW = 512 + 512 + 40 + 40

_prog_cache = {}


def _ap(t, off, dims):
    return bass.AP(t.tensor, t.offset + off, [list(t.ap[0])] + [[s, c] for s, c in dims])


def _split_excess_waits(nc, max_waits=1):
    """This walrus build allows at most one embedded sync-wait per
    instruction; move extra waits onto standalone same-engine NoOps."""
    f = nc.m.functions[0]

    def walk(b):
        yield b
        for sub in getattr(b, "blocks", []) or []:
            yield from walk(sub)

    for top in f.blocks:
        for bb in walk(top):
            insts = getattr(bb, "instructions", None)
            if not insts:
                continue
            new_list = []
            for ins in insts:
                si = ins.sync_info
                waits = list(si.on_wait) if si and si.on_wait else []
                if len(waits) > max_waits:
                    for w in waits[max_waits:]:
                        new_list.append(mybir.InstEventSemaphore(
                            name=f"waitsplit-{nc.next_id()}",
                            ins=[], outs=[], engine=ins.engine,
                            sync_info=mybir.SyncInfo(on_wait=[w], on_update=[]),
                            bass_nofuse=True))
                    ins.sync_info = mybir.SyncInfo(
                        on_wait=waits[:max_waits],
                        on_update=list(si.on_update) if si.on_update else [])
                new_list.append(ins)
            insts[:] = new_list


def _build(split_waits=True):
    nc = bass.Bass()
    npa_d = nc.declare_dram_parameter("npa", [128, NPW], BF16, isOutput=False)
    xh_d = nc.declare_dram_parameter("xh", [128, 2 * XA], BF16, isOutput=False)
    ce_d = nc.declare_dram_parameter("cep", [128, CEW], BF16, isOutput=False)
    wm_d = nc.declare_dram_parameter("wmb", [128, 260], BF16, isOutput=False)
    out_d = nc.declare_dram_parameter("out", [128, NACC], F32, isOutput=True)

    v = nc.vector
    sc = nc.scalar
    gp = nc.gpsimd

    with tile.TileContext(nc) as tc:
        with tc.tile_pool(name="p", bufs=1) as pool, \
             tc.tile_pool(name="ps", bufs=1, space="PSUM") as psp:
            WMB = pool.tile([128, 260], BF16, tag="wmb", name="WMB")
            XS = pool.tile([128, 2 * XA], BF16, tag="xs", name="XS")
            NP_ = pool.tile([128, NPW], BF16, tag="npl", name="NP_")
            CEP = pool.tile([128, CEW], BF16, tag="cep", name="CEP")
            A1 = pool.tile([128, U], BF16, tag="a1", name="A1")
            A2 = pool.tile([128, U], BF16, tag="a2", name="A2")
            EHF = [pool.tile([128, 2 * U], BF16, tag=f"ehf{s}", name=f"EHF{s}")
                   for s in range(NSTEP)]
            GP3 = pool.tile([128, 2 * G * S], BF16, tag="gp3", name="GP3")
            LW = pool.tile([128, 2 * U], F32, tag="lw", name="LW")
            LE = pool.tile([128, U], F32, tag="le", name="LE")
            LE2 = pool.tile([128, U], F32, tag="le2", name="LE2")
            SM = pool.tile([128, 512], F32, tag="sm", name="SM")
            LSE = pool.tile([128, 128], F32, tag="lse", name="LSE")
            EXE = pool.tile([128, 512], BF16, tag="exe", name="EXE")
            EXI = pool.tile([128, G * 10], BF16, tag="exi", name="EXI")
            SI = pool.tile([128, G], F32, tag="si", name="SI")
            S16 = pool.tile([128, 16], F32, tag="s16", name="S16")
            SCR = pool.tile([128, 512], BF16, tag="scr", name="SCR")
            QSC = pool.tile([128, 520], F32, tag="qsc", name="QSC")
            SC2 = pool.tile([128, 64], BF16, tag="sc2", name="SC2")
            XSQ = pool.tile([128, 4224], BF16, tag="xsq", name="XSQ")
            ACC = pool.tile([128, NACC], F32, tag="acc", name="ACC")

            EL = CEP[:, 0:512]
            OHE = CEP[:, 512:1024]
            IL = CEP[:, 1024:1064]
            OHI = CEP[:, 1064:1104]
            WM4 = WMB[:, 0:128]
            WON = WMB[:, 128:256]

            # ---------------- DMAs (single queue, ordered) ----------------
            nc.sync.dma_start(XS[:, 0:XA], xh_d[:, 0:XA])       # chain A
            nc.sync.dma_start(WMB[:], wm_d[:])
            nc.sync.dma_start(XS[:, XA:2 * XA], xh_d[:, XA:2 * XA])  # chain B
            for k in range(4):
                nc.sync.dma_start(NP_[:, k * 2048:(k + 1) * 2048],
                                  npa_d[:, k * 2048:(k + 1) * 2048])
            nc.sync.dma_start(CEP[:], ce_d[:])

            gp.memset(ACC[:], 0.0)
            gp.memset(A1[0:96, :], 1.0)
            gp.memset(A1[96:128, :], 0.0)
            gp.memset(A2[0:96, :], 1.0)
            gp.memset(A2[96:128, :], 0.0)
            # Pool: numerator product chunks (np block k: d-half | oh-half)
            for k in range(4):
                gp.tensor_tensor(GP3[:, k * 1024:(k + 1) * 1024],
                                 NP_[:, k * 2048:k * 2048 + 1024],
                                 NP_[:, k * 2048 + 1024:(k + 1) * 2048],
                                 AL.mult)


            # ---------------- PSUM tiles ----------------
            ps2 = psp.tile([128, 2 * U], F32, tag="rdw", name="ps2")

            # ---------------- ACT: init + exps ----------------
            # chunk-0 exact init: A1[:, q*32] = exp(x_t0 - kappa + start_j)
            sc.activation(_ap(A1[:], 0, [(32, 16)]),
                          _ap(XS[:], 0, [(W, 16)]), AF.Exp)

            def exp_half(s, chain):
                sc.activation(EHF[s][:, chain * U:(chain + 1) * U]
                              .rearrange("p (q c) -> p q c", q=16),
                              _ap(XS[:], chain * XA + s + 1, [(W, 16), (L, 32)]),
                              AF.Exp)

            def exp_full(s):
                sc.activation(_ap(EHF[s][:], 0, [(U, 2), (32, 16), (1, 32)]),
                              _ap(XS[:], s + 1, [(XA, 2), (W, 16), (L, 32)]),
                              AF.Exp)

            KS = 4                       # per-chain exps for s < KS
            for si in range(KS):
                exp_half(si, 0)          # A0..A3
            exp_half(0, 1)               # B0
            for si in range(KS, NSTEP):  # F4,B1,F5,B2,F6,B3,[warmLn],F7,F8
                exp_full(si)
                if si - KS + 1 < KS:
                    exp_half(si - KS + 1, 1)

            sc.activation(EXE[:], EL, AF.Exp)
            sc.activation(EXI[:], IL, AF.Exp)

            # Pool: entity class-quad reduce as 3 strided adds (frees DVE;
            # LSE can then run between the two end-readout Lns on ACT)
            def exq(off):
                return _ap(EXE[:], off, [(4, 128)])
            gp.tensor_tensor(SM[:, 128:256], exq(0), exq(1), AL.add)
            gp.tensor_tensor(SM[:, 256:384], exq(2), exq(3), AL.add)
            gp.tensor_tensor(SM[:, 0:128], SM[:, 128:256], SM[:, 256:384],
                             AL.add)
            gp.tensor_tensor(SCR[:, 0:512], OHE, EL, AL.mult)

            # ---------------- staggered scan ----------------
            LAG = 2
            sched = [("A", i) for i in range(LAG)]
            for i in range(NSTEP):
                sched.append(("B", i))
                if LAG + i < NSTEP:
                    sched.append(("A", LAG + i))
            # step-0 matmuls up front: MM_B0 only needs the A2 memset, so it
            # runs long before xsB lands; MM_A0 right after the init act
            ps_a0 = psp.tile([128, U], F32, tag="mma0", name="psa")
            nc.tensor.matmul(ps_a0[:], WM4, A1[:], start=True, stop=True)
            ps_b0 = psp.tile([128, U], F32, tag="mmb0", name="psb")
            nc.tensor.matmul(ps_b0[:], WM4, A2[:], start=True, stop=True)

            na = nb = 1
            for ei, (ch, st) in enumerate(sched):
                if ch == "A":
                    if st == 0:
                        psx = ps_a0
                    else:
                        psx = psp.tile([128, U], F32, tag=f"mma{na % 2}",
                                       name="psa")
                        na += 1
                        nc.tensor.matmul(psx[:], WM4, A1[:], start=True,
                                         stop=True)
                else:
                    if st == 0:
                        psx = ps_b0
                    else:
                        psx = psp.tile([128, U], F32, tag=f"mmb{nb % 2}",
                                       name="psb")
                        nb += 1
                        nc.tensor.matmul(psx[:], WM4, A2[:], start=True,
                                         stop=True)
                if ch == "A" and st == 1:
                    nc.tensor.matmul(ps2[:, 0:U], WON, A1[:], start=True,
                                     stop=True)
                if ch == "B" and st == 1:
                    nc.tensor.matmul(ps2[:, U:2 * U], WON, A2[:], start=True,
                                     stop=True)
                half = EHF[st][:, 0:U] if ch == "A" else EHF[st][:, U:2 * U]
                v.tensor_tensor(A1[:] if ch == "A" else A2[:], psx[:], half,
                                AL.mult)
                if ch == "B" and st == 2:
                    # warm-readout Ln (ACT reaches it after the exps)
                    sc.activation(LW[:], ps2[:], AF.Ln, accum_out=ACC[:, 1:2])
                if ch == "B" and st == 7:
                    # chain-A end readout overlaps chain B's last step
                    nc.tensor.matmul(ps2[:, 0:U], WON, A1[:], start=True,
                                     stop=True)
                    sc.activation(LE[:], ps2[:, 0:U], AF.Ln,
                                  accum_out=ACC[:, 3:4])
                # emission-score reductions woven into DVE gaps (data-ready
                # by each slot; AP excludes A-pad and the duplicated overlap)
                if ei == 4:
                    v.tensor_scalar(XSQ[0:32, 0:4096],
                                    _ap(XS[0:32, 0:1], 0, [(W, 16), (1, 256)]),
                                    1.0, 0.0, AL.mult, AL.add,
                                    accum_out=ACC[0:32, 0:1])
                if ei == 8:
                    v.tensor_scalar(XSQ[0:32, 0:4224],
                                    _ap(XS[0:32, 0:1], XA, [(W, 16), (1, 264)]),
                                    1.0, 0.0, AL.mult, AL.add,
                                    accum_out=ACC[0:32, 9:10])


            # ---------------- end readouts + tails (run-ahead sorts the
            # actual timing; emission here only sets prefix constraints) ----
            v.tensor_reduce(SI[:],
                            EXI[:].rearrange("p (g c) -> p g c", c=10),
                            axis=AX.X, op=AL.add)
            # scratch outputs target EHF[8]: the WAR on B8's multiplier
            # keeps these ready-early accumulates behind the last scan TT
            v.tensor_scalar(EHF[8][:], GP3[:, 0:1024], 1.0, 0.0,
                            AL.mult, AL.add, accum_out=ACC[:, 10:11])
            v.tensor_scalar(EHF[8][:], GP3[:, 1024:2048], 1.0, 0.0,
                            AL.mult, AL.add, accum_out=ACC[:, 13:14])
            v.tensor_scalar(EHF[8][:], GP3[:, 2048:3072], 1.0, 0.0,
                            AL.mult, AL.add, accum_out=ACC[:, 11:12])
            v.tensor_scalar(EHF[8][:], GP3[:, 3072:4096], 1.0, 0.0,
                            AL.mult, AL.add, accum_out=ACC[:, 14:15])
            v.tensor_scalar(XSQ[:, 0:512], SCR[:, 0:512], 1.0, 0.0,
                            AL.mult, AL.add, accum_out=ACC[:, 6:7])
            v.scalar_tensor_tensor(SC2[:, 0:40], OHI, 1.0, IL, AL.mult,
                                   AL.mult, accum_out=ACC[:, 8:9])
            # entity LSE (SM from Pool); invalid rows contribute ln(4),
            # corrected on host
            sc.activation(LSE[:], SM[:, 0:128], AF.Ln, accum_out=ACC[:, 5:6])
            sc.activation(QSC[:, 516:520], SI[:], AF.Ln,
                          accum_out=ACC[:, 7:8])
            nc.tensor.matmul(ps2[:, U:2 * U], WON, A2[:], start=True,
                             stop=True)
            sc.activation(LE2[:], ps2[:, U:2 * U], AF.Ln,
                          accum_out=ACC[:, 4:5])

            # chunk-0 warm add-back: sum LW chain-A cols {32k}
            v.tensor_scalar(S16[:], _ap(LW[:], 0, [(32, 16)]),
                            1.0, 0.0, AL.mult, AL.add,
                            accum_out=ACC[:, 2:3])
            nc.sync.dma_start(out_d[:], ACC[:])

    if split_waits:
        _split_excess_waits(nc)
    return nc


def _host_planes(inp):
    em = np.asarray(inp["emission_score"], np.float32)
    mask = np.asarray(inp["attention_mask"], bool)
    lbl = np.asarray(inp["seq_labels"], np.int64)
    st = np.asarray(inp["start_transitions"], np.float64)
    en = np.asarray(inp["end_transitions"], np.float64)
    tr = np.asarray(inp["transitions"], np.float64)

    # ---- host-side label/mask-only numerator terms (float64) ----
    lengths = mask.sum(1).astype(np.int64)
    ar = np.arange(B)
    maskf = mask.astype(np.float64)
    trans_sc = np.sum(tr[lbl[:, :-1], lbl[:, 1:]] * maskf[:, 1:])
    se_sc = np.sum(st[lbl[:, 0]]) + np.sum(en[lbl[ar, lengths - 1]])
    sm = float(lengths.sum())

    # ---- natural planes: d1 | oh1 | d2 | oh2 ----
    oh1 = (np.where(mask, lbl, -1) >= 1).astype(np.float32)
    oh2 = (np.where(mask, lbl, -1) >= 2).astype(np.float32)
    def nat(plane):
        t = plane.reshape(NCORES, G, 128, S).astype(BF)
        return t.transpose(0, 2, 1, 3).reshape(NCORES, 128, G * S)

    d1n, o1n = nat(em[:, :, 1] - em[:, :, 0]), nat(oh1)
    d2n, o2n = nat(em[:, :, 2] - em[:, :, 1]), nat(oh2)
    npa = np.empty((NCORES, 128, NPW), BF)
    for k, (dn, on) in enumerate([(d1n, o1n), (d1n, o1n),
                                  (d2n, o2n), (d2n, o2n)]):
        h = slice(1024 * (k % 2), 1024 * (k % 2) + 1024)
        npa[:, :, k * 2048:k * 2048 + 1024] = dn[:, :, h]
        npa[:, :, k * 2048 + 1024:(k + 1) * 2048] = on[:, :, h]

    # ---- scan planes: p = 32j + s%32, free = (s//32)*SP + t ----
    # -kappa folded into the plane (and start_j into col 0) so neither the
    # init act nor the comb exps need a bias operand / the consts DMA
    xs = np.full((B, SP, 4), -40.0 - KAPPA, np.float32)
    for j in range(3):
        xs[:, :S, j] = np.where(mask, em[:, :, j] - KAPPA, -40.0 - KAPPA)
    p3 = np.full((B, SP), 0.0, np.float32)
    p3[:, :S] = np.where(mask, -40.0 - KAPPA, 0.0)
    xs[:, :, 3] = p3
    xs[:, 0, 0:3] += st.astype(np.float32)[None, :]
    xq = (xs.reshape(NCORES, 16, 32, SP, 4).transpose(0, 4, 2, 1, 3)
          .reshape(NCORES, 128, 16, SP).astype(BF))
    # chain-split blocks: A = cols [0:260)+4 pad, B = cols [256:520)
    xh = np.full((NCORES, 128, 2 * XA), BF(-40.0), BF)
    xa = xh[:, :, 0:XA].reshape(NCORES, 128, 16, W)
    xa[:, :, :, 0:260] = xq[:, :, :, 0:260]
    xh[:, :, XA:2 * XA] = xq[:, :, :, 256:520].reshape(NCORES, 128, XA)

    # ---- CE packed planes ----
    elr = np.asarray(inp["entity_logit"], np.float32).reshape(B * 32, 4)
    elab = np.asarray(inp["entity_labels"], np.int64).reshape(-1)
    valid = (elab != 0)
    nvalid = float(valid.sum())
    ohe = np.eye(4, dtype=np.float32)[elab] * valid[:, None]
    il = np.asarray(inp["intent_logit"], np.float32)
    ilab = np.asarray(inp["intent_labels"], np.int64)
    ohi = np.eye(10, dtype=np.float32)[ilab]
    elr = elr * valid[:, None]          # invalid rows -> logits 0 (lse=ln4)
    cep = np.empty((NCORES, 128, CEW), BF)
    # entity rows R: p = R%128, free = (R//128)*4 + c
    elrr = elr.reshape(NCORES, 128, 128, 4)
    cep[:, :, 0:512] = elrr.transpose(0, 2, 1, 3).reshape(NCORES, 128, 512)
    oher = ohe.reshape(NCORES, 128, 128, 4)
    cep[:, :, 512:1024] = oher.transpose(0, 2, 1, 3).reshape(NCORES, 128, 512)
    ilr = il.reshape(NCORES, G, 128, 10)
    cep[:, :, 1024:1064] = ilr.transpose(0, 2, 1, 3).reshape(NCORES, 128, 40)
    ohir = ohi.reshape(NCORES, G, 128, 10)
    cep[:, :, 1064:1104] = ohir.transpose(0, 2, 1, 3).reshape(NCORES, 128, 40)

    # ---- weights bf16 ----
    M4 = np.zeros((4, 4))
    M4[:3, :3] = np.exp(tr)
    M4[:3, 3] = np.exp(en)
    M4[3, 3] = 1.0
    jj = np.arange(128) // 32
    bb = np.arange(128) % 32
    beq = (bb[:, None] == bb[None, :])
    wmb = np.zeros((NCORES, 128, 260), BF)
    wmb[:, :, 0:128] = (M4[jj[:, None], jj[None, :]] * beq).astype(BF)
    wmb[:, :, 128:256] = beq.astype(BF)
    wmb[:, :, 256] = 1.0

    return dict(npa=npa, xh=xh, cep=cep, wmb=wmb, st0=float(st[0]),
                trans_sc=trans_sc, se_sc=se_sc, sm=sm, nvalid=nvalid,
                ninv=float(B * 32) - nvalid)


def kernel(emission_score, attention_mask, seq_labels, entity_logit,
           entity_labels, intent_logit, intent_labels, start_transitions,
           end_transitions, transitions):
    if "nc" not in _prog_cache:
        _prog_cache["nc"] = _build()
    nc = _prog_cache["nc"]

    pl = _host_planes(dict(
        emission_score=emission_score, attention_mask=attention_mask,
        seq_labels=seq_labels, entity_logit=entity_logit,
        entity_labels=entity_labels, intent_logit=intent_logit,
        intent_labels=intent_labels, start_transitions=start_transitions,
        end_transitions=end_transitions, transitions=transitions))

    in_maps = []
    for i in range(NCORES):
        in_maps.append({
            "npa": pl["npa"][i], "xh": pl["xh"][i], "cep": pl["cep"][i],
            "wmb": pl["wmb"][i],
        })
    res = run_bass_kernel_spmd(nc, in_maps, core_ids=list(range(NCORES)))
    acc = np.zeros(NACC, np.float64)
    for r in res.results:
        acc += np.asarray(r["out"], np.float64).sum(0)
    qsum = acc[0] + acc[9] + acc[10] + acc[11] + acc[13] + acc[14]

    sm = pl["sm"]
    # undo the folded fills on xs ch0: dead/virtual cells hold the
    # bf16-ROUNDED constant bf16(-40-kappa), alive cells em0-kappa,
    # col 0 carries +start_0
    cdead = float(BF(-40.0 - KAPPA))
    q = qsum + KAPPA * sm - B * pl["st0"] - cdead * (B * SP - sm)
    score = q + pl["trans_sc"] + pl["se_sc"]
    den = (acc[3] + acc[4] - acc[1] + acc[2]) / 4.0 + KAPPA * sm
    loss1 = (den - score) / B
    loss2 = (acc[5] - math.log(4.0) * pl["ninv"] - acc[6]) \
        / max(pl["nvalid"], 1.0)
    loss3 = (acc[7] - acc[8]) / B
    loss = (loss1 + loss2 + loss3) / 3.0
    return np.stack([loss, loss1, loss2, loss3]).astype(np.float32)
